# revision 60
# baseline (speedup 1.0000x reference)
"""GAT (2-layer, 8-head) Trainium2 Bass kernel — v3.

Data-parallel over batch: 16 graphs -> 8 cores x 2 graphs each. No collectives.

Math (dense reformulation, engine-balanced):
  - Edge softmax + scatter collapse to dense [N,N] ops via the host-built
    log-count matrix:  pun[src,dst] = exp(leaky_relu(el[src]+er[dst], 0.2)
    + log(count)), with -60 for absent edges (exp underflows fp8 to 0), so
    the mask multiply folds into the score exp.
  - elu via the max identity  elu(x) + 1 = max(x+1, min(exp(x), 1))
    (e^x >= 1+x everywhere), so the tail per (head, node-tile) is ONE ACT
    exp plus fused DVE ops:
        e = Exp(rp*rec)               (ACT, per-partition scale)
        u = rp*rec + 1                (DVE tensor_scalar or ACT Identity)
        v = (e min 1) max u           (DVE scalar_tensor_tensor)
        acc += v                      (gpsimd f32 tensor_tensor)
    The per-head -1 and the /8 head-mean fold into the layer tails.
  - Denominators are computed EARLY: per head/node-tile a tiny DoubleRow
    matmul of pun against a +128 constant column right after attention,
    then one batched reciprocal per (graph, head-half) -> rec_all, so the
    per-unit e/u ops never wait on a reciprocal chain.
  - Scale folding: h-tiles carry 16x values, W carries 8x; both quantize
    to fp8 e4m3 in the normal range.  rst PSUM is then 128x and
    rec = 1/(128*denom) makes u = rp*rec + 1 exact.  Scores are 16x with
    a 1/16 folded into the el/er copies (leaky_relu is positively
    homogeneous).
  - feat AND rst matmuls run fp8 e4m3 with MatmulPerfMode.DoubleRow (0.5
    cycles/row): feat pairs k-chunks (contraction 256/mm); rst pairs the
    two src node-tiles.  el/er matmuls stay bf16.
  - Attention per (g, src-tile, 4-head half): el added as a per-partition
    scalar AP via small DVE tensor_scalar ops, then one wide Prelu, one
    Pool log-mask add, one wide Exp straight into fp8 pun.  Head-halves
    are staggered into the feat stream.  er rows bounce through DRAM and
    return as ONE broadcast DMA per (layer, graph).
  - xm node-major (the residual, minus 1) comes from transposed matmuls
    (lhsT = xr) emitted in the layer-0 tail to fill the layer boundary.
  - The fp8 weight stream is mostly SBUF-resident up front (no buffer
    rotation stalls); rst tails run at high scheduler priority.

Layouts per core (nodes padded 207->256, two 128-row node tiles per graph):
  h{0,1}Tb [128, 6k, 2g, 256n] bf16 (16x)   feat-transposed activations
  h{0,1}_8 [128, 6k, 2g, 256n] fp8  (16x)   same, for DoubleRow matmuls
  feat     [128, 2g, 2sc, 8h, 768] fp8 (128x)
  pun8     [128, 2g, 2sc, 8*208] fp8        unnormalized attention
  acc      [128, 2g, 2dt, 768] f32          sum_h (elu_h + 1)
"""
import math
import ml_dtypes
import numpy as np

B, C_IN, N, T = 16, 2, 207, 12
EMB = 64
HEADS = 8
F = EMB * T            # 768
HF = HEADS * F         # 6144
NC_COUNT = 8
GPC = B // NC_COUNT    # graphs per core
NP = 256               # padded nodes per graph
KC = F // 128          # 6 contraction chunks (bf16); 3 DoubleRow pairs
FO_CH = HF // 512      # 12 fo chunks
NC1 = N + 1            # 208 dst columns (col 207 = padding, mask 0)

_BUILT = None
_LAST = None


def _build(dbg=False):
    import contextlib

    import concourse.mybir as mybir
    import concourse.tile as tile
    from concourse import bacc
    from concourse.masks import make_identity

    F32 = mybir.dt.float32
    BF16 = mybir.dt.bfloat16
    FP8 = mybir.dt.float8e4

    AF = mybir.ActivationFunctionType
    OP = mybir.AluOpType
    DR = mybir.MatmulPerfMode.DoubleRow

    nc = bacc.Bacc("TRN2", target_bir_lowering=False, debug=False)

    xr_d = nc.dram_tensor("xr", [24, GPC * NP], BF16, kind="ExternalInput")
    wmain_d = nc.dram_tensor("wmain", [2, FO_CH, 128, KC * 512], FP8,
                             kind="ExternalInput")
    wlr_d = nc.dram_tensor("wlr", [2, 128, KC * 16], BF16, kind="ExternalInput")
    wpre_d = nc.dram_tensor("wpre", [24, 2 * 2 * F], BF16, kind="ExternalInput")
    maskt_d = nc.dram_tensor("maskt", [128, 2, HEADS * NC1], BF16,
                             kind="ExternalInput")
    out_d = nc.dram_tensor("outp", [GPC, NP, F], F32, kind="ExternalOutput")
    if dbg:
        dbg_h0Tb = nc.dram_tensor("dbg_h0Tb", [128, KC, GPC, NP], F32,
                                  kind="ExternalOutput")
        dbg_h0n = nc.dram_tensor("dbg_h0n", [128, 4, F], F32,
                                 kind="ExternalOutput")
        dbg_feat = nc.dram_tensor("dbg_feat", [128, GPC, 2, HEADS, 770], F32,
                                  kind="ExternalOutput")
        dbg_pun = nc.dram_tensor("dbg_pun", [128, 2, HEADS * NC1], F32,
                                 kind="ExternalOutput")
        dbg_rst = nc.dram_tensor("dbg_rst", [128, 770], F32,
                                 kind="ExternalOutput")
        dbg_acc = nc.dram_tensor("dbg_acc", [128, GPC, 2, F], F32,
                                 kind="ExternalOutput")
        dbg_h1Tb = nc.dram_tensor("dbg_h1Tb", [128, KC, GPC, NP], F32,
                                  kind="ExternalOutput")

    def mm(out, lhsT, rhs, start, stop, **kw):
        nc.tensor.matmul(out, lhsT, rhs, start=start, stop=stop, **kw)

    # chunk index after which head h's rst is emitted (right after the
    # chunk whose drains complete the head's feat columns)
    rst_after = {}
    for h in range(HEADS):
        c_need = math.ceil((h + 1) * F / 512)
        rst_after.setdefault(c_need - 1, []).append(h)

    with tile.TileContext(nc, pool_alloc_mode="queue") as tc:
        with contextlib.ExitStack() as ctx:
            big = ctx.enter_context(tc.tile_pool(name="big", bufs=1))
            wpool = ctx.enter_context(tc.tile_pool(name="wpool", bufs=18))
            s8p = ctx.enter_context(tc.tile_pool(name="s8p", bufs=2))
            ebsp = ctx.enter_context(tc.tile_pool(name="ebsp", bufs=2))
            etp = ctx.enter_context(tc.tile_pool(name="etp", bufs=2))
            utp = ctx.enter_context(tc.tile_pool(name="utp", bufs=2))
            vtp = ctx.enter_context(tc.tile_pool(name="vtp", bufs=2))
            ps = ctx.enter_context(tc.tile_pool(name="ps", bufs=2, space="PSUM"))
            ps2 = ctx.enter_context(tc.tile_pool(name="ps2", bufs=2,
                                                 space="PSUM"))
            tmpp = ctx.enter_context(tc.tile_pool(name="tmpp", bufs=2))
            psf = ctx.enter_context(tc.tile_pool(name="psf", bufs=2, space="PSUM"))
            dram = ctx.enter_context(tc.tile_pool(name="dram", bufs=1, space="DRAM"))

            # ---- persistent tiles ----
            h0Tb = big.tile([128, KC, GPC, NP], BF16, tag="h0Tb")
            h1Tb = big.tile([128, KC, GPC, NP], BF16, tag="h1Tb")
            h0_8 = big.tile([128, KC, GPC, NP], FP8, tag="h08")
            h1_8 = big.tile([128, KC, GPC, NP], FP8, tag="h18")
            feat = big.tile([128, GPC, 2, HEADS, F], FP8, tag="feat")
            pun8 = big.tile([128, GPC, 2, HEADS * NC1], FP8, tag="pun8")
            mask8 = big.tile([128, 2, HEADS * NC1], BF16, tag="mask8")
            acc = big.tile([128, GPC, 2, F], F32, tag="acc")
            h0nm1 = big.tile([128, 4, F], BF16, tag="h0nm1")
            el_sb = big.tile([128, GPC, 2, 8], F32, tag="el")
            wlr_sb = big.tile([128, 2, KC, 16], BF16, tag="wlr")
            ident = big.tile([128, 128], BF16, tag="ident")
            neg16 = big.tile([128, 1], F32, tag="neg16")
            ones128 = big.tile([128, 2, 1], FP8, tag="ones128")
            rec_all = big.tile([128, GPC, 16], F32, tag="recall")
            er_dr = dram.tile([2, GPC, 8, NC1], BF16, tag="erd")

            import concourse.bass as bass_mod

            def copy_on(eng, out, in_):
                if eng is nc.scalar:
                    nc.scalar.activation(out, in_, AF.Identity)
                else:
                    eng.tensor_copy(out, in_)

            # round-robin engine picker for the feat PSUM->SBUF drains
            _cp = {"i": 0}

            def drain_copy(out, in_):
                # gpsimd cannot access PSUM; DVE-leaning DVE/ACT alternation
                seq = [nc.vector, nc.scalar, nc.vector]
                e = seq[_cp["i"] % len(seq)]
                _cp["i"] += 1
                copy_on(e, out, in_)

            with nc.named_scope("pre"):
                xr = big.tile([24, GPC, NP], BF16, tag="xr")
                wpre = big.tile([24, 4, F], BF16, tag="wpre")
                nc.sync.dma_start(mask8, maskt_d.ap())
                nc.sync.dma_start(wpre, wpre_d.ap())
                nc.sync.dma_start(xr, xr_d.ap())
                nc.sync.dma_start(wlr_sb[:, 0], wlr_d.ap()[0])
                nc.sync.dma_start(wlr_sb[:, 1], wlr_d.ap()[1])
                # fp8 weight stream mostly resident up front: no buffer
                # rotation stalls, and layer-1 rarely waits.  The last 4
                # layer-1 chunks ride the SWDGE queue inline (ring reuses
                # slots of long-consumed layer-0 chunks).
                wts = {}
                for wl, wc in [(0, b) for b in range(6)]:
                    wt = wpool.tile([128, KC, 512], FP8, tag="wst")
                    nc.sync.dma_start(wt, wmain_d.ap()[wl, wc])
                    wts[(wl, wc)] = wt
                make_identity(nc, ident)
                nc.vector.memset(neg16, -16.0)
                # +128 constant column for the denominator matmuls
                nc.gpsimd.memset(ones128, 128.0)
                # h1 pad columns (never written by the layer-0 tail)
                for g in range(GPC):
                    nc.gpsimd.memset(h1Tb[:, :, g, N:NP], 0.0)
                    nc.gpsimd.memset(h1_8[:, :, g, N:NP], 0.0)

                # h0Tb/h0_8 [(e t), n]: 16x activations (wpre 16x block)
                for g in range(GPC):
                    for mt in range(KC):
                        ps_s = ps.tile([128, NP], F32, tag="rstps")
                        ps_c = ps.tile([128, NP], F32, tag="rstps")
                        mm(ps_s, wpre[:, 0, mt * 128:(mt + 1) * 128],
                           xr[:, g, :], True, True)
                        mm(ps_c, wpre[:, 1, mt * 128:(mt + 1) * 128],
                           xr[:, g, :], True, True)
                        t01 = tmpp.tile([128, NP], BF16, tag="t01")
                        nc.scalar.activation(t01, ps_c, AF.Prelu, alpha=0.01)
                        nc.vector.tensor_tensor(h0Tb[:, mt, g, :], t01, ps_s,
                                                OP.add)
                        nc.gpsimd.tensor_copy(h0_8[:, mt, g, :],
                                              h0Tb[:, mt, g, :])

            def emit_h0n():
                # xm node-major minus 1 (the residual; only consumed in the
                # layer-1 tail): emitted mid layer-0 to fill idle slots
                for g in range(GPC):
                    for nt in range(2):
                        ps_ns = ps.tile([128, F], F32, tag="rstps")
                        ps_nc = ps.tile([128, F], F32, tag="rstps")
                        lhs = xr[:, g, nt * 128:(nt + 1) * 128]
                        for cs, cw in ((0, 512), (512, 256)):
                            mm(ps_ns[:, cs:cs + cw], lhs,
                               wpre[:, 2, cs:cs + cw], True, True)
                            mm(ps_nc[:, cs:cs + cw], lhs,
                               wpre[:, 3, cs:cs + cw], True, True)
                        t0n = tmpp.tile([128, F], BF16, tag="t0n")
                        nc.scalar.activation(t0n, ps_nc, AF.Prelu, alpha=0.01)
                        nc.vector.scalar_tensor_tensor(
                            h0nm1[:, g * 2 + nt, :], ps_ns, -1.0, t0n,
                            OP.add, OP.add)

            # ---- two GAT layers ----
            for l in range(2):
                hTb = h0Tb if l == 0 else h1Tb
                h8 = h0_8 if l == 0 else h1_8

                with nc.named_scope(f"layer{l}_head"):
                    # el (node-partitioned, 16x) and er rows -> DRAM bounce
                    for g in range(GPC):
                        for nt in range(2):
                            elp = ps2.tile([128, 8], F32, tag="smallps")
                            for k in range(KC):
                                mm(elp, hTb[:, k, g, nt * 128:(nt + 1) * 128],
                                   wlr_sb[:, l, k, 0:8], k == 0, k == KC - 1)
                            # 1/16: el_sb holds true-scale el
                            nc.scalar.activation(el_sb[:, g, nt, :],
                                                 elp, AF.Identity,
                                                 scale=0.0625)
                        # er-only matmul so the rows land at partitions 0:8
                        ertp = ps2.tile([8, NP], F32, tag="smallps")
                        for k in range(KC):
                            mm(ertp, wlr_sb[:, l, k, 8:16], hTb[:, k, g, :],
                               k == 0, k == KC - 1)
                        er_bf = tmpp.tile([8, NC1], BF16, tag="erbf")
                        nc.scalar.activation(er_bf, ertp[:, 0:NC1],
                                             AF.Identity, scale=0.0625)
                        nc.sync.dma_start(er_dr[l, g], er_bf)

                    # er broadcast loads (one per graph, all heads)
                    ebps = []
                    for g in range(GPC):
                        ebp = ebsp.tile([128, 8, NC1], BF16, tag="ebs")
                        src = er_dr[l, g]
                        nc.sync.dma_start(
                            ebp, bass_mod.AP(tensor=src.tensor,
                                             offset=src.offset,
                                             ap=[[0, 128], [NC1, 8], [1, NC1]]))
                        ebps.append(ebp)
                    if l == 0:
                        # remaining resident weights, behind the attention
                        # DMAs in SP order so layer 0 starts promptly
                        for wl, wc in [(a, b) for a in range(2)
                                       for b in range(FO_CH)
                                       if (0, 6) <= (a, b) < (1, 6)]:
                            wt = wpool.tile([128, KC, 512], FP8, tag="wst")
                            nc.sync.dma_start(wt, wmain_d.ap()[wl, wc])
                            wts[(wl, wc)] = wt

                def att_half(g, sc, hh, l=l):
                    """scores+exp+mask for heads [4*hh, 4*hh+4) of (g, sc)."""
                    lo, hi = 4 * hh * NC1, (4 * hh + 4) * NC1
                    s8 = s8p.tile([128, 4 * NC1], BF16, tag="s8")
                    for h in range(4 * hh, 4 * hh + 4):
                        nc.vector.tensor_scalar_add(
                            s8[:, (h - 4 * hh) * NC1:(h - 4 * hh + 1) * NC1],
                            ebps[g][:, h, :],
                            el_sb[:, g, sc, h:h + 1])
                    nc.scalar.activation(s8, s8, AF.Prelu, alpha=0.2)
                    # log-count mask: pun = exp(lrelu(s) + log(count))
                    nc.gpsimd.tensor_tensor(s8, s8, mask8[:, sc, lo:hi],
                                            OP.add)
                    nc.scalar.activation(pun8[:, g, sc, lo:hi], s8, AF.Exp)

                def do_rst(h, l=l):
                    """rst matmuls + normalize + elu(max identity) + accum."""
                    hp = tc.high_priority(offset=400)
                    hp.__enter__()
                    for g in range(GPC):
                        for dt in range(2):
                            dw = 128 if dt == 0 else N - 128
                            dwm = 128 if dt == 0 else 80
                            rp = ps.tile([128, F], F32, tag="rstps")
                            # fp8 DoubleRow: the two src tiles are the two
                            # row planes; one mm per PSUM-bank region
                            for cs, cw in ((0, 512), (512, 256)):
                                dsl = pun8[:, g, :,
                                           h * NC1 + dt * 128:
                                           h * NC1 + dt * 128 + dwm]
                                mm(rp[0:dwm, cs:cs + cw],
                                   dsl, feat[:, g, :, h, cs:cs + cw],
                                   True, True, perf_mode=DR)
                            rec = rec_all[0:dw, g, h * 2 + dt:h * 2 + dt + 1]
                            et = etp.tile([128, F], BF16, tag="et")
                            nc.scalar.activation(et[0:dw], rp[0:dw, 0:768],
                                                 AF.Exp, scale=rec)
                            ut = utp.tile([128, F], BF16, tag="ut")
                            if h % 2 == 0:
                                nc.vector.tensor_scalar(
                                    ut[0:dw], rp[0:dw, 0:768],
                                    rec, 1.0, OP.mult, OP.add)
                            else:
                                nc.scalar.activation(ut[0:dw], rp[0:dw, 0:768],
                                                     AF.Identity,
                                                     scale=rec,
                                                     bias=1.0)
                            a = acc[0:dw, g, dt, :]
                            if h == 0:
                                nc.vector.scalar_tensor_tensor(
                                    a, et[0:dw], 1.0, ut[0:dw],
                                    OP.min, OP.max)
                            else:
                                vt = vtp.tile([128, F], BF16, tag="vt")
                                nc.vector.scalar_tensor_tensor(
                                    vt[0:dw], et[0:dw], 1.0, ut[0:dw],
                                    OP.min, OP.max)
                                nc.gpsimd.tensor_tensor(a, a, vt[0:dw], OP.add)
                    hp.__exit__(None, None, None)


                def do_den(g, hh, dentiles, l=l):
                    """denominators for heads [4hh,4hh+4) -> den psum; then
                    one batched reciprocal into rec_all."""
                    den = dentiles[g]
                    for h in range(4 * hh, 4 * hh + 4):
                        for dt in range(2):
                            dwm = 128 if dt == 0 else 80
                            dsl = pun8[:, g, :,
                                       h * NC1 + dt * 128:
                                       h * NC1 + dt * 128 + dwm]
                            mm(den[0:dwm, h * 2 + dt:h * 2 + dt + 1],
                               dsl, ones128, True, True, perf_mode=DR)
                    nc.vector.reciprocal(
                        rec_all[:, g, 8 * hh:8 * hh + 8],
                        den[:, 8 * hh:8 * hh + 8])

                # first att halves up front (rst h0 fires after c=1)
                dentiles = []
                for _dg in range(GPC):
                    dent = ps2.tile([128, 16], F32, tag="smallps", name=f"den{_dg}")
                    dentiles.append(dent)
                for g in range(GPC):
                    for sc in range(2):
                        att_half(g, sc, 0)
                for g in range(GPC):
                    do_den(g, 0, dentiles)

                # feat matmul stream (fp8 DoubleRow), rst interleaved per head
                with nc.named_scope(f"layer{l}_main"):
                    for c in range(FO_CH):
                        if c == 2:   # second att halves, one chunk of slack
                            for g in range(GPC):
                                for sc in range(2):
                                    att_half(g, sc, 1)
                            for g in range(GPC):
                                do_den(g, 1, dentiles)
                        if (l, c) in wts:
                            wt = wts[(l, c)]
                        else:
                            wt = wpool.tile([128, KC, 512], FP8, tag="wst")
                            nc.gpsimd.dma_start(wt, wmain_d.ap()[l, c])
                        for g in range(GPC):
                            for nt in range(2):
                                fp = psf.tile([128, 512], F32, tag="featps")
                                for kk in range(KC // 2):
                                    mm(fp,
                                       h8[:, 2 * kk:2 * kk + 2, g,
                                          nt * 128:(nt + 1) * 128],
                                       wt[:, 2 * kk:2 * kk + 2, :],
                                       kk == 0, kk == KC // 2 - 1,
                                       perf_mode=DR)
                                lo = c * 512
                                while lo < (c + 1) * 512:
                                    hh, off = lo // F, lo % F
                                    ln = min((c + 1) * 512 - lo, F - off)
                                    drain_copy(
                                        feat[:, g, nt, hh, off:off + ln],
                                        fp[:, lo - c * 512:lo - c * 512 + ln])
                                    lo += ln
                        for h in rst_after.get(c, ()):
                            do_rst(h)

                # layer tail
                with nc.named_scope(f"layer{l}_tail"):
                    if l == 0:
                        for g in range(GPC):
                            for dt in range(2):
                                dw = 128 if dt == 0 else N - 128
                                hn = tmpp.tile([128, F], BF16, tag="hn")
                                # 16*(0.125*acc - 1) = 2*acc - 16  (16x h1)
                                nc.scalar.activation(hn, acc[:, g, dt, :],
                                                     AF.Identity,
                                                     scale=2.0,
                                                     bias=neg16[:, 0:1])
                                # 6 transposes into one psum tile, one
                                # strided drain each for bf16 and fp8
                                tpb = ps2.tile([128, KC, 128], BF16,
                                               tag="smallps")
                                for k in range(KC):
                                    nc.tensor.transpose(
                                        tpb[:, k, :],
                                        hn[:, k * 128:(k + 1) * 128],
                                        ident)
                                eng = nc.vector if dt else nc.scalar
                                copy_on(
                                    eng,
                                    h1Tb[:, :, g, dt * 128:dt * 128 + dw],
                                    tpb[:, :, 0:dw])
                                nc.gpsimd.tensor_copy(
                                    h1_8[:, :, g, dt * 128:dt * 128 + dw],
                                    h1Tb[:, :, g, dt * 128:dt * 128 + dw])
                        emit_h0n()
                        if dbg:
                            dbt = tmpp.tile([128, KC, GPC, NP], F32, tag="db1")
                            nc.vector.tensor_copy(dbt, h1Tb)
                            nc.sync.dma_start(dbg_h1Tb.ap(), dbt)
                            dba = tmpp.tile([128, GPC, 2, F], F32, tag="dba")
                            nc.vector.tensor_copy(dba, acc)
                            nc.sync.dma_start(dbg_acc.ap(), dba)
                    else:
                        for g in range(GPC):
                            for dt in range(2):
                                dw = 128 if dt == 0 else N - 128
                                ot = tmpp.tile([128, F], F32, tag="ot")
                                # out = xm + gc = h0nm1 + 0.125*acc
                                nc.vector.scalar_tensor_tensor(
                                    ot[0:dw], acc[0:dw, g, dt, :], 0.125,
                                    h0nm1[0:dw, g * 2 + dt, :],
                                    OP.mult, OP.add)
                                nc.sync.dma_start(
                                    out_d.ap()[g, dt * 128:dt * 128 + dw, :],
                                    ot[0:dw])

    nc.compile()
    return nc


def _host_prep(inputs):
    """Shard + preprocess the full inputs into per-core in_maps."""
    x = np.asarray(inputs["x"], dtype=np.float32)
    src = np.asarray(inputs["src"]).astype(np.int64)
    dst = np.asarray(inputs["dst"]).astype(np.int64)
    Ws = np.asarray(inputs["Ws"], dtype=np.float64)
    Wc = np.asarray(inputs["Wc"], dtype=np.float64)
    W1 = np.asarray(inputs["W1"], dtype=np.float64)
    W2 = np.asarray(inputs["W2"], dtype=np.float64)
    al1 = np.asarray(inputs["al1"], dtype=np.float64)
    ar1 = np.asarray(inputs["ar1"], dtype=np.float64)
    al2 = np.asarray(inputs["al2"], dtype=np.float64)
    ar2 = np.asarray(inputs["ar2"], dtype=np.float64)

    # xr: [B, 24, NP] = x[b, c, n, t] -> [(c t), n], node-padded with zeros
    xr = np.zeros((B, 24, NP), np.float32)
    xr[:, :, :N] = x.transpose(0, 1, 3, 2).reshape(B, 24, N)

    # wmain: [2, 12, 128, 6*512] fp8 = 8*W[k*128+p, c*512 + (kk? no:
    # w8[l, c, p, k, j] = 8*W_l[k*128+p, c*512+j]
    wm = np.stack([W1, W2]).astype(np.float32) * 8.0          # [2, 768, 6144]
    wm = wm.reshape(2, KC, 128, FO_CH, 512).transpose(0, 3, 2, 1, 4)
    wmain = np.ascontiguousarray(
        wm.reshape(2, FO_CH, 128, KC * 512)).astype(ml_dtypes.float8_e4m3fn)

    def fuse(W, al, ar):
        Wh = W.reshape(F, HEADS, F)
        wl = np.einsum("khf,hf->kh", Wh, al)
        wr = np.einsum("khf,hf->kh", Wh, ar)
        return np.concatenate([wl, wr], axis=1).astype(np.float32)  # [F, 16]

    wlr = np.stack([fuse(W1, al1, ar1), fuse(W2, al2, ar2)])  # [2, 768, 16]
    wlr = wlr.reshape(2, KC, 128, 16).transpose(0, 2, 1, 3)
    wlr = np.ascontiguousarray(
        wlr.reshape(2, 128, KC * 16)).astype(ml_dtypes.bfloat16)

    # wpre [24, 4, 768]: blocks [16x s | 16x c | 1x s | 1x c]
    # wpret[ct, conv*F + f] = delta(t, f%T) * W[f//T, c]
    wpret = np.zeros((24, 2, F), np.float32)
    for conv, W in ((0, Ws), (1, Wc)):
        Wf = W.astype(np.float32)
        for t in range(T):
            for c in range(C_IN):
                wpret[c * T + t, conv, t::T] = Wf[:, c]
    wpre = np.concatenate([16.0 * wpret, wpret], axis=1)  # [24, 4, 768]
    wpre = wpre.reshape(24, 4 * F).astype(ml_dtypes.bfloat16)

    # maskt [128, 2, 8*208]: log(count(src -> dst)) per head; -60 for no edge
    # (exp(x - 60) underflows fp8 to exactly 0)
    maskt = np.zeros((128, 2, NC1), np.float32)
    np.add.at(maskt, (src % 128, src // 128, dst), 1.0)
    maskt = np.where(maskt > 0, np.log(np.maximum(maskt, 1e-9)), -60.0)
    maskt = np.tile(maskt[:, :, None, :].astype(np.float32),
                    (1, 1, HEADS, 1))
    maskt = maskt.reshape(128, 2, HEADS * NC1).astype(ml_dtypes.bfloat16)

    shared = dict(wmain=wmain, wlr=wlr, wpre=wpre, maskt=maskt)
    in_maps = []
    for core in range(NC_COUNT):
        m = dict(shared)
        xrc = xr[core * GPC:(core + 1) * GPC]           # [GPC, 24, NP]
        xrc = xrc.transpose(1, 0, 2).reshape(24, GPC * NP)
        m["xr"] = np.ascontiguousarray(xrc).astype(ml_dtypes.bfloat16)
        in_maps.append(m)
    return in_maps


def kernel(**inputs):
    global _BUILT, _LAST
    from concourse.bass_utils import run_bass_kernel_spmd

    if _BUILT is None:
        _BUILT = _build()
    nc = _BUILT

    in_maps = _host_prep(inputs)
    res = run_bass_kernel_spmd(nc, in_maps, core_ids=list(range(NC_COUNT)))
    _LAST = res

    out = np.empty((B, EMB, N, T), np.float32)
    for core in range(NC_COUNT):
        o = res.results[core]["outp"]  # [GPC, NP, F]
        o = o[:, :N, :].reshape(GPC, N, EMB, T).transpose(0, 2, 1, 3)
        out[core * GPC:(core + 1) * GPC] = o
    return out


# revision 61
# speedup vs baseline: 1.0076x; 1.0076x over previous
"""GAT (2-layer, 8-head) Trainium2 Bass kernel — v3.

Data-parallel over batch: 16 graphs -> 8 cores x 2 graphs each. No collectives.

Math (dense reformulation, engine-balanced):
  - Edge softmax + scatter collapse to dense [N,N] ops via the host-built
    log-count matrix:  pun[src,dst] = exp(leaky_relu(el[src]+er[dst], 0.2)
    + log(count)), with -60 for absent edges (exp underflows fp8 to 0), so
    the mask multiply folds into the score exp.
  - elu via the max identity  elu(x) + 1 = max(x+1, min(exp(x), 1))
    (e^x >= 1+x everywhere), so the tail per (head, node-tile) is ONE ACT
    exp plus fused DVE ops:
        e = Exp(rp*rec)               (ACT, per-partition scale)
        u = rp*rec + 1                (DVE tensor_scalar or ACT Identity)
        v = (e min 1) max u           (DVE scalar_tensor_tensor)
        acc += v                      (gpsimd f32 tensor_tensor)
    The per-head -1 and the /8 head-mean fold into the layer tails.
  - Denominators are computed EARLY: per head/node-tile a tiny DoubleRow
    matmul of pun against a +128 constant column right after attention,
    then one batched reciprocal per (graph, head-half) -> rec_all, so the
    per-unit e/u ops never wait on a reciprocal chain.
  - Scale folding: h-tiles carry 16x values, W carries 8x; both quantize
    to fp8 e4m3 in the normal range.  rst PSUM is then 128x and
    rec = 1/(128*denom) makes u = rp*rec + 1 exact.  Scores are 16x with
    a 1/16 folded into the el/er copies (leaky_relu is positively
    homogeneous).
  - feat AND rst matmuls run fp8 e4m3 with MatmulPerfMode.DoubleRow (0.5
    cycles/row): feat pairs k-chunks (contraction 256/mm); rst pairs the
    two src node-tiles.  el/er matmuls stay bf16.
  - Attention per (g, src-tile, 4-head half): el added as a per-partition
    scalar AP via small DVE tensor_scalar ops, then one wide Prelu, one
    Pool log-mask add, one wide Exp straight into fp8 pun.  Head-halves
    are staggered into the feat stream.  er rows bounce through DRAM and
    return as ONE broadcast DMA per (layer, graph).
  - xm node-major (the residual, minus 1) comes from transposed matmuls
    (lhsT = xr) emitted in the layer-0 tail to fill the layer boundary.
  - The fp8 weight stream is mostly SBUF-resident up front (no buffer
    rotation stalls); rst tails run at high scheduler priority.

Layouts per core (nodes padded 207->256, two 128-row node tiles per graph):
  h{0,1}Tb [128, 6k, 2g, 256n] bf16 (16x)   feat-transposed activations
  h{0,1}_8 [128, 6k, 2g, 256n] fp8  (16x)   same, for DoubleRow matmuls
  feat     [128, 2g, 2sc, 8h, 768] fp8 (128x)
  pun8     [128, 2g, 2sc, 8*208] fp8        unnormalized attention
  acc      [128, 2g, 2dt, 768] f32          sum_h (elu_h + 1)
"""
import math
import ml_dtypes
import numpy as np

B, C_IN, N, T = 16, 2, 207, 12
EMB = 64
HEADS = 8
F = EMB * T            # 768
HF = HEADS * F         # 6144
NC_COUNT = 8
GPC = B // NC_COUNT    # graphs per core
NP = 256               # padded nodes per graph
KC = F // 128          # 6 contraction chunks (bf16); 3 DoubleRow pairs
FO_CH = HF // 512      # 12 fo chunks
NC1 = N + 1            # 208 dst columns (col 207 = padding, mask 0)

_BUILT = None
_LAST = None


def _build(dbg=False):
    import contextlib

    import concourse.mybir as mybir
    import concourse.tile as tile
    from concourse import bacc
    from concourse.masks import make_identity

    F32 = mybir.dt.float32
    BF16 = mybir.dt.bfloat16
    FP8 = mybir.dt.float8e4

    AF = mybir.ActivationFunctionType
    OP = mybir.AluOpType
    DR = mybir.MatmulPerfMode.DoubleRow

    nc = bacc.Bacc("TRN2", target_bir_lowering=False, debug=False)

    xr_d = nc.dram_tensor("xr", [24, GPC * NP], BF16, kind="ExternalInput")
    wmain_d = nc.dram_tensor("wmain", [2, FO_CH, 128, KC * 512], FP8,
                             kind="ExternalInput")
    wlr_d = nc.dram_tensor("wlr", [2, 128, KC * 16], BF16, kind="ExternalInput")
    wpre_d = nc.dram_tensor("wpre", [24, 2 * 2 * F], BF16, kind="ExternalInput")
    maskt_d = nc.dram_tensor("maskt", [128, 2, HEADS * NC1], BF16,
                             kind="ExternalInput")
    out_d = nc.dram_tensor("outp", [GPC, NP, F], F32, kind="ExternalOutput")
    if dbg:
        dbg_h0Tb = nc.dram_tensor("dbg_h0Tb", [128, KC, GPC, NP], F32,
                                  kind="ExternalOutput")
        dbg_h0n = nc.dram_tensor("dbg_h0n", [128, 4, F], F32,
                                 kind="ExternalOutput")
        dbg_feat = nc.dram_tensor("dbg_feat", [128, GPC, 2, HEADS, 770], F32,
                                  kind="ExternalOutput")
        dbg_pun = nc.dram_tensor("dbg_pun", [128, 2, HEADS * NC1], F32,
                                 kind="ExternalOutput")
        dbg_rst = nc.dram_tensor("dbg_rst", [128, 770], F32,
                                 kind="ExternalOutput")
        dbg_acc = nc.dram_tensor("dbg_acc", [128, GPC, 2, F], F32,
                                 kind="ExternalOutput")
        dbg_h1Tb = nc.dram_tensor("dbg_h1Tb", [128, KC, GPC, NP], F32,
                                  kind="ExternalOutput")

    def mm(out, lhsT, rhs, start, stop, **kw):
        nc.tensor.matmul(out, lhsT, rhs, start=start, stop=stop, **kw)

    # chunk index after which head h's rst is emitted (right after the
    # chunk whose drains complete the head's feat columns)
    rst_after = {}
    for h in range(HEADS):
        c_need = math.ceil((h + 1) * F / 512)
        rst_after.setdefault(c_need - 1, []).append(h)

    with tile.TileContext(nc, pool_alloc_mode="queue") as tc:
        with contextlib.ExitStack() as ctx:
            big = ctx.enter_context(tc.tile_pool(name="big", bufs=1))
            wpool = ctx.enter_context(tc.tile_pool(name="wpool", bufs=18))
            s8p = ctx.enter_context(tc.tile_pool(name="s8p", bufs=2))
            ebsp = ctx.enter_context(tc.tile_pool(name="ebsp", bufs=2))
            etp = ctx.enter_context(tc.tile_pool(name="etp", bufs=2))
            utp = ctx.enter_context(tc.tile_pool(name="utp", bufs=2))
            vtp = ctx.enter_context(tc.tile_pool(name="vtp", bufs=2))
            ps = ctx.enter_context(tc.tile_pool(name="ps", bufs=2, space="PSUM"))
            ps2 = ctx.enter_context(tc.tile_pool(name="ps2", bufs=2,
                                                 space="PSUM"))
            tmpp = ctx.enter_context(tc.tile_pool(name="tmpp", bufs=2))
            psf = ctx.enter_context(tc.tile_pool(name="psf", bufs=2, space="PSUM"))
            dram = ctx.enter_context(tc.tile_pool(name="dram", bufs=1, space="DRAM"))

            # ---- persistent tiles ----
            h0Tb = big.tile([128, KC, GPC, NP], BF16, tag="h0Tb")
            h1Tb = big.tile([128, KC, GPC, NP], BF16, tag="h1Tb")
            h0_8 = big.tile([128, KC, GPC, NP], FP8, tag="h08")
            h1_8 = big.tile([128, KC, GPC, NP], FP8, tag="h18")
            feat = big.tile([128, GPC, 2, HEADS * F], FP8, tag="feat")
            pun8 = big.tile([128, GPC, 2, HEADS * NC1], FP8, tag="pun8")
            mask8 = big.tile([128, 2, HEADS * NC1], BF16, tag="mask8")
            acc = big.tile([128, GPC, 2, F], F32, tag="acc")
            h0nm1 = big.tile([128, 4, F], BF16, tag="h0nm1")
            el_sb = big.tile([128, GPC, 2, 8], F32, tag="el")
            wlr_sb = big.tile([128, 2, KC, 16], BF16, tag="wlr")
            ident = big.tile([128, 128], BF16, tag="ident")
            neg16 = big.tile([128, 1], F32, tag="neg16")
            ones128 = big.tile([128, 2, 1], FP8, tag="ones128")
            rec_all = big.tile([128, GPC, 16], F32, tag="recall")
            er_dr = dram.tile([2, GPC, 8, NC1], BF16, tag="erd")

            import concourse.bass as bass_mod

            def copy_on(eng, out, in_):
                if eng is nc.scalar:
                    nc.scalar.activation(out, in_, AF.Identity)
                else:
                    eng.tensor_copy(out, in_)

            # round-robin engine picker for the feat PSUM->SBUF drains
            _cp = {"i": 0}

            def drain_copy(out, in_):
                # gpsimd cannot access PSUM; DVE-leaning DVE/ACT alternation
                seq = [nc.vector, nc.scalar, nc.vector]
                e = seq[_cp["i"] % len(seq)]
                _cp["i"] += 1
                copy_on(e, out, in_)

            with nc.named_scope("pre"):
                xr = big.tile([24, GPC, NP], BF16, tag="xr")
                wpre = big.tile([24, 4, F], BF16, tag="wpre")
                nc.sync.dma_start(mask8, maskt_d.ap())
                nc.sync.dma_start(wpre, wpre_d.ap())
                nc.sync.dma_start(xr, xr_d.ap())
                nc.sync.dma_start(wlr_sb[:, 0], wlr_d.ap()[0])
                nc.sync.dma_start(wlr_sb[:, 1], wlr_d.ap()[1])
                # fp8 weight stream mostly resident up front: no buffer
                # rotation stalls, and layer-1 rarely waits.  The last 4
                # layer-1 chunks ride the SWDGE queue inline (ring reuses
                # slots of long-consumed layer-0 chunks).
                wts = {}
                for wl, wc in [(0, b) for b in range(6)]:
                    wt = wpool.tile([128, KC, 512], FP8, tag="wst")
                    nc.sync.dma_start(wt, wmain_d.ap()[wl, wc])
                    wts[(wl, wc)] = wt
                make_identity(nc, ident)
                nc.vector.memset(neg16, -16.0)
                # +128 constant column for the denominator matmuls
                nc.gpsimd.memset(ones128, 128.0)
                # h1 pad columns (never written by the layer-0 tail)
                for g in range(GPC):
                    nc.gpsimd.memset(h1Tb[:, :, g, N:NP], 0.0)
                    nc.gpsimd.memset(h1_8[:, :, g, N:NP], 0.0)

                # h0Tb/h0_8 [(e t), n]: 16x activations (wpre 16x block)
                for g in range(GPC):
                    for mt in range(KC):
                        ps_s = ps.tile([128, NP], F32, tag="rstps")
                        ps_c = ps.tile([128, NP], F32, tag="rstps")
                        mm(ps_s, wpre[:, 0, mt * 128:(mt + 1) * 128],
                           xr[:, g, :], True, True)
                        mm(ps_c, wpre[:, 1, mt * 128:(mt + 1) * 128],
                           xr[:, g, :], True, True)
                        t01 = tmpp.tile([128, NP], BF16, tag="t01")
                        nc.scalar.activation(t01, ps_c, AF.Prelu, alpha=0.01)
                        nc.vector.tensor_tensor(h0Tb[:, mt, g, :], t01, ps_s,
                                                OP.add)
                        nc.gpsimd.tensor_copy(h0_8[:, mt, g, :],
                                              h0Tb[:, mt, g, :])

            def emit_h0n():
                # xm node-major minus 1 (the residual; only consumed in the
                # layer-1 tail): emitted mid layer-0 to fill idle slots
                for g in range(GPC):
                    for nt in range(2):
                        ps_ns = ps.tile([128, F], F32, tag="rstps")
                        ps_nc = ps.tile([128, F], F32, tag="rstps")
                        lhs = xr[:, g, nt * 128:(nt + 1) * 128]
                        for cs, cw in ((0, 512), (512, 256)):
                            mm(ps_ns[:, cs:cs + cw], lhs,
                               wpre[:, 2, cs:cs + cw], True, True)
                            mm(ps_nc[:, cs:cs + cw], lhs,
                               wpre[:, 3, cs:cs + cw], True, True)
                        t0n = tmpp.tile([128, F], BF16, tag="t0n")
                        nc.scalar.activation(t0n, ps_nc, AF.Prelu, alpha=0.01)
                        nc.vector.scalar_tensor_tensor(
                            h0nm1[:, g * 2 + nt, :], ps_ns, -1.0, t0n,
                            OP.add, OP.add)

            # ---- two GAT layers ----
            for l in range(2):
                hTb = h0Tb if l == 0 else h1Tb
                h8 = h0_8 if l == 0 else h1_8

                with nc.named_scope(f"layer{l}_head"):
                    # el (node-partitioned, 16x) and er rows -> DRAM bounce
                    for g in range(GPC):
                        for nt in range(2):
                            elp = ps2.tile([128, 8], F32, tag="smallps")
                            for k in range(KC):
                                mm(elp, hTb[:, k, g, nt * 128:(nt + 1) * 128],
                                   wlr_sb[:, l, k, 0:8], k == 0, k == KC - 1)
                            # 1/16: el_sb holds true-scale el
                            nc.scalar.activation(el_sb[:, g, nt, :],
                                                 elp, AF.Identity,
                                                 scale=0.0625)
                        # er-only matmul so the rows land at partitions 0:8
                        ertp = ps2.tile([8, NP], F32, tag="smallps")
                        for k in range(KC):
                            mm(ertp, wlr_sb[:, l, k, 8:16], hTb[:, k, g, :],
                               k == 0, k == KC - 1)
                        er_bf = tmpp.tile([8, NC1], BF16, tag="erbf")
                        nc.scalar.activation(er_bf, ertp[:, 0:NC1],
                                             AF.Identity, scale=0.0625)
                        nc.sync.dma_start(er_dr[l, g], er_bf)

                    # er broadcast loads (one per graph, all heads)
                    ebps = []
                    for g in range(GPC):
                        ebp = ebsp.tile([128, 8, NC1], BF16, tag="ebs")
                        src = er_dr[l, g]
                        nc.sync.dma_start(
                            ebp, bass_mod.AP(tensor=src.tensor,
                                             offset=src.offset,
                                             ap=[[0, 128], [NC1, 8], [1, NC1]]))
                        ebps.append(ebp)
                    if l == 0:
                        # remaining resident weights, behind the attention
                        # DMAs in SP order so layer 0 starts promptly
                        for wl, wc in [(a, b) for a in range(2)
                                       for b in range(FO_CH)
                                       if (0, 6) <= (a, b) < (1, 6)]:
                            wt = wpool.tile([128, KC, 512], FP8, tag="wst")
                            nc.sync.dma_start(wt, wmain_d.ap()[wl, wc])
                            wts[(wl, wc)] = wt

                def att_half(g, sc, hh, l=l):
                    """scores+exp+mask for heads [4*hh, 4*hh+4) of (g, sc)."""
                    lo, hi = 4 * hh * NC1, (4 * hh + 4) * NC1
                    s8 = s8p.tile([128, 4 * NC1], BF16, tag="s8")
                    for h in range(4 * hh, 4 * hh + 4):
                        nc.vector.tensor_scalar_add(
                            s8[:, (h - 4 * hh) * NC1:(h - 4 * hh + 1) * NC1],
                            ebps[g][:, h, :],
                            el_sb[:, g, sc, h:h + 1])
                    nc.scalar.activation(s8, s8, AF.Prelu, alpha=0.2)
                    # log-count mask: pun = exp(lrelu(s) + log(count))
                    nc.gpsimd.tensor_tensor(s8, s8, mask8[:, sc, lo:hi],
                                            OP.add)
                    nc.scalar.activation(pun8[:, g, sc, lo:hi], s8, AF.Exp)

                def do_rst(h, l=l):
                    """rst matmuls + normalize + elu(max identity) + accum."""
                    hp = tc.high_priority(offset=400)
                    hp.__enter__()
                    for g in range(GPC):
                        for dt in range(2):
                            dw = 128 if dt == 0 else N - 128
                            dwm = 128 if dt == 0 else 80
                            rp = ps.tile([128, F], F32, tag="rstps")
                            # fp8 DoubleRow: the two src tiles are the two
                            # row planes; one mm per PSUM-bank region
                            for cs, cw in ((0, 512), (512, 256)):
                                dsl = pun8[:, g, :,
                                           h * NC1 + dt * 128:
                                           h * NC1 + dt * 128 + dwm]
                                mm(rp[0:dwm, cs:cs + cw],
                                   dsl,
                                   feat[:, g, :, h * F + cs:h * F + cs + cw],
                                   True, True, perf_mode=DR)
                            rec = rec_all[0:dw, g, h * 2 + dt:h * 2 + dt + 1]
                            et = etp.tile([128, F], BF16, tag="et")
                            nc.scalar.activation(et[0:dw], rp[0:dw, 0:768],
                                                 AF.Exp, scale=rec)
                            ut = utp.tile([128, F], BF16, tag="ut")
                            if h % 2 == 0:
                                nc.vector.tensor_scalar(
                                    ut[0:dw], rp[0:dw, 0:768],
                                    rec, 1.0, OP.mult, OP.add)
                            else:
                                nc.scalar.activation(ut[0:dw], rp[0:dw, 0:768],
                                                     AF.Identity,
                                                     scale=rec,
                                                     bias=1.0)
                            a = acc[0:dw, g, dt, :]
                            if h == 0:
                                nc.vector.scalar_tensor_tensor(
                                    a, et[0:dw], 1.0, ut[0:dw],
                                    OP.min, OP.max)
                            else:
                                vt = vtp.tile([128, F], BF16, tag="vt")
                                nc.vector.scalar_tensor_tensor(
                                    vt[0:dw], et[0:dw], 1.0, ut[0:dw],
                                    OP.min, OP.max)
                                nc.gpsimd.tensor_tensor(a, a, vt[0:dw], OP.add)
                    hp.__exit__(None, None, None)


                def do_den(g, hh, dentiles, l=l):
                    """denominators for heads [4hh,4hh+4) -> den psum; then
                    one batched reciprocal into rec_all."""
                    den = dentiles[g]
                    for h in range(4 * hh, 4 * hh + 4):
                        for dt in range(2):
                            dwm = 128 if dt == 0 else 80
                            dsl = pun8[:, g, :,
                                       h * NC1 + dt * 128:
                                       h * NC1 + dt * 128 + dwm]
                            mm(den[0:dwm, h * 2 + dt:h * 2 + dt + 1],
                               dsl, ones128, True, True, perf_mode=DR)
                    nc.vector.reciprocal(
                        rec_all[:, g, 8 * hh:8 * hh + 8],
                        den[:, 8 * hh:8 * hh + 8])

                # first att halves up front (rst h0 fires after c=1)
                dentiles = []
                for _dg in range(GPC):
                    dent = ps2.tile([128, 16], F32, tag="smallps", name=f"den{_dg}")
                    dentiles.append(dent)
                for g in range(GPC):
                    for sc in range(2):
                        att_half(g, sc, 0)
                for g in range(GPC):
                    do_den(g, 0, dentiles)

                # feat matmul stream (fp8 DoubleRow), rst interleaved per head
                with nc.named_scope(f"layer{l}_main"):
                    for c in range(FO_CH):
                        if c == 2:   # second att halves, one chunk of slack
                            for g in range(GPC):
                                for sc in range(2):
                                    att_half(g, sc, 1)
                            for g in range(GPC):
                                do_den(g, 1, dentiles)
                        if (l, c) in wts:
                            wt = wts[(l, c)]
                        else:
                            wt = wpool.tile([128, KC, 512], FP8, tag="wst")
                            nc.gpsimd.dma_start(wt, wmain_d.ap()[l, c])
                        for g in range(GPC):
                            for nt in range(2):
                                fp = psf.tile([128, 512], F32, tag="featps")
                                for kk in range(KC // 2):
                                    mm(fp,
                                       h8[:, 2 * kk:2 * kk + 2, g,
                                          nt * 128:(nt + 1) * 128],
                                       wt[:, 2 * kk:2 * kk + 2, :],
                                       kk == 0, kk == KC // 2 - 1,
                                       perf_mode=DR)
                                drain_copy(
                                    feat[:, g, nt, c * 512:(c + 1) * 512],
                                    fp)
                        for h in rst_after.get(c, ()):
                            do_rst(h)

                # layer tail
                with nc.named_scope(f"layer{l}_tail"):
                    if l == 0:
                        for g in range(GPC):
                            for dt in range(2):
                                dw = 128 if dt == 0 else N - 128
                                hn = tmpp.tile([128, F], BF16, tag="hn")
                                # 16*(0.125*acc - 1) = 2*acc - 16  (16x h1)
                                nc.scalar.activation(hn, acc[:, g, dt, :],
                                                     AF.Identity,
                                                     scale=2.0,
                                                     bias=neg16[:, 0:1])
                                # 6 transposes into one psum tile, one
                                # strided drain each for bf16 and fp8
                                tpb = ps2.tile([128, KC, 128], BF16,
                                               tag="smallps")
                                for k in range(KC):
                                    nc.tensor.transpose(
                                        tpb[:, k, :],
                                        hn[:, k * 128:(k + 1) * 128],
                                        ident)
                                eng = nc.vector if dt else nc.scalar
                                copy_on(
                                    eng,
                                    h1Tb[:, :, g, dt * 128:dt * 128 + dw],
                                    tpb[:, :, 0:dw])
                                nc.gpsimd.tensor_copy(
                                    h1_8[:, :, g, dt * 128:dt * 128 + dw],
                                    h1Tb[:, :, g, dt * 128:dt * 128 + dw])
                        emit_h0n()
                        if dbg:
                            dbt = tmpp.tile([128, KC, GPC, NP], F32, tag="db1")
                            nc.vector.tensor_copy(dbt, h1Tb)
                            nc.sync.dma_start(dbg_h1Tb.ap(), dbt)
                            dba = tmpp.tile([128, GPC, 2, F], F32, tag="dba")
                            nc.vector.tensor_copy(dba, acc)
                            nc.sync.dma_start(dbg_acc.ap(), dba)
                    else:
                        for g in range(GPC):
                            for dt in range(2):
                                dw = 128 if dt == 0 else N - 128
                                ot = tmpp.tile([128, F], F32, tag="ot")
                                # out = xm + gc = h0nm1 + 0.125*acc
                                nc.vector.scalar_tensor_tensor(
                                    ot[0:dw], acc[0:dw, g, dt, :], 0.125,
                                    h0nm1[0:dw, g * 2 + dt, :],
                                    OP.mult, OP.add)
                                nc.sync.dma_start(
                                    out_d.ap()[g, dt * 128:dt * 128 + dw, :],
                                    ot[0:dw])

    nc.compile()
    return nc


def _host_prep(inputs):
    """Shard + preprocess the full inputs into per-core in_maps."""
    x = np.asarray(inputs["x"], dtype=np.float32)
    src = np.asarray(inputs["src"]).astype(np.int64)
    dst = np.asarray(inputs["dst"]).astype(np.int64)
    Ws = np.asarray(inputs["Ws"], dtype=np.float64)
    Wc = np.asarray(inputs["Wc"], dtype=np.float64)
    W1 = np.asarray(inputs["W1"], dtype=np.float64)
    W2 = np.asarray(inputs["W2"], dtype=np.float64)
    al1 = np.asarray(inputs["al1"], dtype=np.float64)
    ar1 = np.asarray(inputs["ar1"], dtype=np.float64)
    al2 = np.asarray(inputs["al2"], dtype=np.float64)
    ar2 = np.asarray(inputs["ar2"], dtype=np.float64)

    # xr: [B, 24, NP] = x[b, c, n, t] -> [(c t), n], node-padded with zeros
    xr = np.zeros((B, 24, NP), np.float32)
    xr[:, :, :N] = x.transpose(0, 1, 3, 2).reshape(B, 24, N)

    # wmain: [2, 12, 128, 6*512] fp8 = 8*W[k*128+p, c*512 + (kk? no:
    # w8[l, c, p, k, j] = 8*W_l[k*128+p, c*512+j]
    wm = np.stack([W1, W2]).astype(np.float32) * 8.0          # [2, 768, 6144]
    wm = wm.reshape(2, KC, 128, FO_CH, 512).transpose(0, 3, 2, 1, 4)
    wmain = np.ascontiguousarray(
        wm.reshape(2, FO_CH, 128, KC * 512)).astype(ml_dtypes.float8_e4m3fn)

    def fuse(W, al, ar):
        Wh = W.reshape(F, HEADS, F)
        wl = np.einsum("khf,hf->kh", Wh, al)
        wr = np.einsum("khf,hf->kh", Wh, ar)
        return np.concatenate([wl, wr], axis=1).astype(np.float32)  # [F, 16]

    wlr = np.stack([fuse(W1, al1, ar1), fuse(W2, al2, ar2)])  # [2, 768, 16]
    wlr = wlr.reshape(2, KC, 128, 16).transpose(0, 2, 1, 3)
    wlr = np.ascontiguousarray(
        wlr.reshape(2, 128, KC * 16)).astype(ml_dtypes.bfloat16)

    # wpre [24, 4, 768]: blocks [16x s | 16x c | 1x s | 1x c]
    # wpret[ct, conv*F + f] = delta(t, f%T) * W[f//T, c]
    wpret = np.zeros((24, 2, F), np.float32)
    for conv, W in ((0, Ws), (1, Wc)):
        Wf = W.astype(np.float32)
        for t in range(T):
            for c in range(C_IN):
                wpret[c * T + t, conv, t::T] = Wf[:, c]
    wpre = np.concatenate([16.0 * wpret, wpret], axis=1)  # [24, 4, 768]
    wpre = wpre.reshape(24, 4 * F).astype(ml_dtypes.bfloat16)

    # maskt [128, 2, 8*208]: log(count(src -> dst)) per head; -60 for no edge
    # (exp(x - 60) underflows fp8 to exactly 0)
    maskt = np.zeros((128, 2, NC1), np.float32)
    np.add.at(maskt, (src % 128, src // 128, dst), 1.0)
    maskt = np.where(maskt > 0, np.log(np.maximum(maskt, 1e-9)), -60.0)
    maskt = np.tile(maskt[:, :, None, :].astype(np.float32),
                    (1, 1, HEADS, 1))
    maskt = maskt.reshape(128, 2, HEADS * NC1).astype(ml_dtypes.bfloat16)

    shared = dict(wmain=wmain, wlr=wlr, wpre=wpre, maskt=maskt)
    in_maps = []
    for core in range(NC_COUNT):
        m = dict(shared)
        xrc = xr[core * GPC:(core + 1) * GPC]           # [GPC, 24, NP]
        xrc = xrc.transpose(1, 0, 2).reshape(24, GPC * NP)
        m["xr"] = np.ascontiguousarray(xrc).astype(ml_dtypes.bfloat16)
        in_maps.append(m)
    return in_maps


def kernel(**inputs):
    global _BUILT, _LAST
    from concourse.bass_utils import run_bass_kernel_spmd

    if _BUILT is None:
        _BUILT = _build()
    nc = _BUILT

    in_maps = _host_prep(inputs)
    res = run_bass_kernel_spmd(nc, in_maps, core_ids=list(range(NC_COUNT)))
    _LAST = res

    out = np.empty((B, EMB, N, T), np.float32)
    for core in range(NC_COUNT):
        o = res.results[core]["outp"]  # [GPC, NP, F]
        o = o[:, :N, :].reshape(GPC, N, EMB, T).transpose(0, 2, 1, 3)
        out[core * GPC:(core + 1) * GPC] = o
    return out
